# revision 7
# baseline (speedup 1.0000x reference)
"""Trainium2 Bass kernel for nn_NetworkActivity_layer (masked linear):

    out = x @ (weight * mask.T).T + bias      x:(4096,15000) w:(500,15000)
                                              mask:(15000,500) bias:(500,)

Strategy: shard the contraction (gene) dim K=15000 across 8 NeuronCores
(1875 genes/core, padded to 1920 = 15 k-tiles of 128; the extra row at
gene 1875 carries the bias via an all-ones x column). Each core computes
a partial (4096,500) output; the host sums the 8 partials.

Numerics: the masked weights mw = weight * mask.T are premultiplied on
the host and scaled by 2^14 (so the fp8 tail tiles land in e4m3's normal
range); the host divides the summed output by 2^14. The first NKB
k-tiles run in bf16; the last NF k-tiles run as NF/2 fp8e4 DoubleRow
matmuls (two 128-row k-tiles per 211ns PE pass = 2x rate). Exact rel
err vs the fp32 reference on the real inputs: 1.25e-2 for NF=2,
1.75e-2 for NF=4 (gate 2e-2) — measured with a bit-exact host sim.

Per-core operands (host-packed):
  xt:  (32, 128, NKB*128) bf16  xt[m, p, k*128+c] = xpad[m*128+c, k*128+p]
       stationary lhsT slices (K=128 genes, M=128 batch), two ~2KB-row
       half loads per tile.
  gh:  (128, 6*NKB*128) bf16    the first GRP=6 m-tiles repacked
       k-tile-major: gh[p, (k*6 + m)*128 + c] = xt[m, p, k*128+c]; one
       DMA per k-tile (1.5KB rows) so arrivals match consumption order.
  x8:  (8, 128, 4*NF, 128) fp8  x8[q, p, mi*NF+2*dri+ks, c] =
       xpad[(4q+mi)*128+c, (NKB+2*dri+ks)*128+p]; DoubleRow stationary
       [K=128, ks=2, M=128] slices for 4 m-tiles per DMA.
  mw:  (128, NKB*500) bf16      mw[p, k*500+n] = mwpad[k*128+p, n];
       k0 rides the sync ring (starts earlier), k1.. on scalar, one DMA
       per k-tile so the PE never waits on a big chunk.
  mw8: (128, NF, 500) fp8       mw8[p, kk, n] = mwpad[(NKB+kk)*128+p, n]
  out: (32, 128, 500) fp16      PSUM fp32, cast on the PSUM->SBUF copy;
       host upcasts, sums, descales.

Head scheduling: the first GRP=6 m-tiles are held in 6 PSUM banks and
their matmuls are emitted k-tile-outer / m-inner, so every arriving
(mw_k, gh_k) pair unlocks 6 matmuls (1.28us of PE work per ~0.9us of
arrivals) — the whole weight load and the 1.2->2.4GHz HAM clock ramp
are absorbed by useful work. Junk matmuls on a memset tile bridge the
framework preamble to the first arrival so the ramp window never
resets. All DMAs keep >=1KB contiguous rows (sub-KB rows run ~20GB/s).
"""

import functools
import os

import ml_dtypes
import numpy as np

B, G, P = 4096, 15000, 500
N_CORES = 8
GS = G // N_CORES          # 1875 genes per core
KT = 128                   # k-tile size (partition dim; 128 enables FWL)
NK = 15                    # k-tiles per core
NF = 2                     # fp8 k-tiles (must be even); rel err 1.25e-2
NKB = NK - NF              # bf16 k-tiles
NDR = NF // 2              # DoubleRow matmuls per m-tile
KP = NK * KT               # 1920 padded genes (row GS=1875 carries bias)
MT = 128                   # batch tile
NM = B // MT               # 32 batch tiles
NQ = NM // 4               # fp8 x quads (4 m-tiles per DMA)
SCALE = np.float32(2.0 ** 14)

GRP = 6                    # m-tiles resident in PSUM during the mw load
XT_HALVES = [(0, NKB // 2 + 1), (NKB // 2 + 1, NKB)]
N_JUNK = 5                 # PE clock prewarm matmuls

_BF16 = ml_dtypes.bfloat16
_F8 = ml_dtypes.float8_e4m3

LAST_EXEC_TIME_NS = None
LAST_TRACE = None
LAST_RESULTS = None


def _install_profshim():
    """Make run_bass_kernel_spmd(trace=True) work in the axon container:
    recreate the antenv.axon_hooks NTFF hook + keep artifacts local."""
    import sys
    import types

    if "antenv.axon_hooks" not in sys.modules:
        import antenv
        from trn_agent_boot.trn_boot import _ntff_profile_via_ctypes

        mod = types.ModuleType("antenv.axon_hooks")
        mod._hook = _ntff_profile_via_ctypes("/opt/axon/libaxon_pjrt.so")
        mod.set_axon_ntff_profile_hook = lambda h: setattr(mod, "_hook", h)
        mod.get_axon_ntff_profile_hook = lambda: mod._hook
        sys.modules["antenv.axon_hooks"] = mod
        antenv.axon_hooks = mod

    import concourse.bass_utils as bu

    bu.upload_artifacts = lambda tmpdir: f"file://{tmpdir}"


@functools.lru_cache(maxsize=1)
def _build():
    import concourse.bass as bass
    import concourse.mybir as mybir
    import concourse.tile as tile
    from concourse import bacc

    nc = bacc.Bacc(
        "TRN2", target_bir_lowering=False, debug=False, num_devices=N_CORES
    )
    bf16 = mybir.dt.bfloat16
    f16 = mybir.dt.float16
    f32 = mybir.dt.float32
    f8 = mybir.dt.float8e4
    DR = mybir.MatmulPerfMode.DoubleRow

    xt_d = nc.dram_tensor("xt", [NM, KT, NKB * MT], bf16, kind="ExternalInput")
    gh_d = nc.dram_tensor("gh", [KT, GRP * NKB * MT], bf16, kind="ExternalInput")
    x8_d = nc.dram_tensor("x8", [NQ, KT, 4 * NF * MT], f8, kind="ExternalInput")
    mw_d = nc.dram_tensor("mw", [KT, NKB * P], bf16, kind="ExternalInput")
    mw8_d = nc.dram_tensor("mw8", [KT, NF * P], f8, kind="ExternalInput")
    out_d = nc.dram_tensor("out", [NM, MT, P], f16, kind="ExternalOutput")

    with tile.TileContext(nc) as tc:
        with (
            tc.tile_pool(name="jpool", bufs=1) as jpool,
            tc.tile_pool(name="wpool", bufs=1) as wpool,
            tc.tile_pool(name="xpool", bufs=1) as xpool,
            tc.tile_pool(name="opool", bufs=4) as opool,
            tc.tile_pool(name="pspool", bufs=1, space=bass.MemorySpace.PSUM) as pspool,
        ):
            # Clock prewarm: PE activity right after the framework preamble
            # so the 1.2->2.4GHz ramp window elapses while the first mw
            # k-tile + gh k-tile are still in flight.
            junk = jpool.tile([KT, 512], bf16)
            nc.gpsimd.memset(junk[:], 0.0)
            jps = pspool.tile([MT, 512], f32, tag="ps", bufs=2)
            for _ in range(N_JUNK):
                nc.tensor.matmul(jps[:], junk[:, 0:128], junk[:], start=True, stop=True)

            # Premultiplied masked weights: one DMA per k-tile so matmul k
            # never waits on a multi-tile chunk. k0 on the sync ring (it
            # starts ~0.7us earlier than scalar); the rest on scalar.
            mw = wpool.tile([KT, NKB * P], bf16)
            nc.sync.dma_start(mw[:, 0:P], mw_d[:, 0:P])
            for k in range(1, NKB):
                nc.scalar.dma_start(
                    mw[:, k * P : (k + 1) * P], mw_d[:, k * P : (k + 1) * P]
                )
            mw8 = wpool.tile([KT, NF, P], f8)
            nc.scalar.dma_start(mw8[:, :, :], mw8_d[:, :])

            # Head group: GRP m-tiles pinned in PSUM, matmuls emitted
            # k-tile-outer / m-inner; gh is packed k-tile-major so each
            # per-k DMA lands exactly when that k's matmuls are due.
            gh = xpool.tile([KT, GRP * NKB * MT], bf16, name="gh", tag="gh", bufs=1)
            for k in range(NKB):
                nc.sync.dma_start(
                    gh[:, k * GRP * MT : (k + 1) * GRP * MT],
                    gh_d[:, k * GRP * MT : (k + 1) * GRP * MT],
                )
            x8q0 = xpool.tile([KT, 4 * NF, MT], f8, tag="x8", bufs=3, name="x8q0")
            nc.sync.dma_start(x8q0[:, :, :], x8_d[0])
            x8q1 = xpool.tile([KT, 4 * NF, MT], f8, tag="x8", bufs=3, name="x8q1")
            nc.sync.dma_start(x8q1[:, :, :], x8_d[1])
            gps = [
                pspool.tile([MT, P], f32, name=f"gps{m}", tag=f"gps{m}", bufs=1)
                for m in range(GRP)
            ]
            for k in range(NKB):
                for m in range(GRP):
                    nc.tensor.matmul(
                        gps[m][:],
                        gh[:, (k * GRP + m) * MT : (k * GRP + m + 1) * MT],
                        mw[:, k * P : (k + 1) * P],
                        start=(k == 0),
                        stop=False,
                    )
            for m in range(GRP):
                x8q = x8q0 if m < 4 else x8q1
                mi = m % 4
                for dri in range(NDR):
                    nc.tensor.matmul(
                        gps[m][:],
                        x8q[:, mi * NF + 2 * dri : mi * NF + 2 * dri + 2, :],
                        mw8[:, 2 * dri : 2 * dri + 2, :],
                        start=False,
                        stop=(dri == NDR - 1),
                        perf_mode=DR,
                    )
            for m in range(GRP):
                ot = opool.tile([MT, P], f16, tag="ot", name="ot")
                nc.vector.tensor_copy(ot[:], gps[m][:])
                nc.scalar.dma_start(out_d[m], ot[:])

            # Steady state: one m-tile at a time, mw fully resident.
            x8q = x8q1
            for m in range(GRP, NM):
                if m % 4 == 0 and m // 4 >= 2:
                    x8q = xpool.tile([KT, 4 * NF, MT], f8, tag="x8", bufs=3, name="x8q")
                    nc.sync.dma_start(x8q[:, :, :], x8_d[m // 4])
                xt = xpool.tile([KT, NKB * MT], bf16, tag="xt", bufs=8, name="xt")
                for a, b in XT_HALVES:
                    nc.sync.dma_start(
                        xt[:, a * MT : b * MT], xt_d[m][:, a * MT : b * MT]
                    )
                ps = pspool.tile([MT, P], f32, tag="ps", bufs=2, name="ps")
                for k in range(NKB):
                    nc.tensor.matmul(
                        ps[:],
                        xt[:, k * MT : (k + 1) * MT],
                        mw[:, k * P : (k + 1) * P],
                        start=(k == 0),
                        stop=False,
                    )
                mi = m % 4
                for dri in range(NDR):
                    nc.tensor.matmul(
                        ps[:],
                        x8q[:, mi * NF + 2 * dri : mi * NF + 2 * dri + 2, :],
                        mw8[:, 2 * dri : 2 * dri + 2, :],
                        start=False,
                        stop=(dri == NDR - 1),
                        perf_mode=DR,
                    )
                ot = opool.tile([MT, P], f16, tag="ot", name="ot")
                nc.vector.tensor_copy(ot[:], ps[:])
                nc.scalar.dma_start(out_d[m], ot[:])
    nc.compile()
    return nc


def _pack_inputs(x, weight, mask, bias):
    """Host-side shard + pre-tile. Returns in_maps for the 8 cores."""
    xb = np.asarray(x, dtype=np.float32).astype(_BF16)  # (B, G) one cast pass
    wf = np.asarray(weight, dtype=np.float32)
    mf = np.asarray(mask, dtype=np.float32)
    bf = np.asarray(bias, dtype=np.float32)

    in_maps = []
    for core in range(N_CORES):
        g0 = core * GS
        xpad = np.zeros((B, KP), dtype=_BF16)
        xpad[:, :GS] = xb[:, g0 : g0 + GS]
        xpad[:, GS] = _BF16(1.0)  # bias column
        # bf16 k-tiles: [m, c, k, p] -> [m, p, k, c]
        xt = np.ascontiguousarray(
            xpad[:, : NKB * KT].reshape(NM, MT, NKB, KT).transpose(0, 3, 2, 1)
        ).reshape(NM, KT, NKB * MT)
        # head-group region repacked k-tile-major: [p, k, m, c]
        gh = np.ascontiguousarray(
            xt[:GRP].reshape(GRP, KT, NKB, MT).transpose(1, 2, 0, 3)
        ).reshape(KT, GRP * NKB * MT)
        # fp8 k-tiles: [q, mi, c, kk, p] -> [q, p, mi, kk, c]
        x8 = np.ascontiguousarray(
            xpad[:, NKB * KT :].astype(_F8)
            .reshape(NQ, 4, MT, NF, KT)
            .transpose(0, 4, 1, 3, 2)
        ).reshape(NQ, KT, 4 * NF * MT)

        # premultiplied masked weights, scaled into fp8 range:
        # mwpad[g, n] = w[n, g0+g] * mask[g0+g, n] * 2^14
        mwpad = np.zeros((KP, P), dtype=np.float32)
        mwpad[:GS] = wf[:, g0 : g0 + GS].T * mf[g0 : g0 + GS]
        if core == 0:
            mwpad[GS] = bf  # bias row (counted exactly once across cores)
        mwpad *= SCALE
        mwt = np.ascontiguousarray(
            mwpad[: NKB * KT].reshape(NKB, KT, P).transpose(1, 0, 2)
        ).reshape(KT, NKB * P).astype(_BF16)
        mw8 = np.ascontiguousarray(
            mwpad[NKB * KT :].reshape(NF, KT, P).transpose(1, 0, 2)
        ).reshape(KT, NF * P).astype(_F8)
        in_maps.append({"xt": xt, "x8": x8, "gh": gh, "mw": mwt, "mw8": mw8})
    return in_maps


def kernel(x, weight, mask, bias):
    global LAST_EXEC_TIME_NS, LAST_TRACE, LAST_RESULTS

    profile = bool(int(os.environ.get("KERNEL_PROFILE", "0")))
    if profile:
        _install_profshim()

    nc = _build()
    in_maps = _pack_inputs(x, weight, mask, bias)

    from concourse.bass_utils import run_bass_kernel_spmd

    tmpdir = None
    if profile:
        import tempfile

        base = os.environ.get("KERNEL_TRACE_DIR")
        if base:
            os.makedirs(base, exist_ok=True)
        tmpdir = tempfile.mkdtemp(prefix="ktrace_", dir=base)

    res = run_bass_kernel_spmd(
        nc,
        in_maps,
        core_ids=list(range(N_CORES)),
        trace=profile,
        tmpdir=tmpdir,
    )
    LAST_EXEC_TIME_NS = res.exec_time_ns
    LAST_TRACE = (
        res.instructions_and_trace[1] if res.instructions_and_trace else None
    )
    LAST_RESULTS = res

    parts = np.stack(
        [r["out"].astype(np.float32).reshape(B, P) for r in res.results]
    )
    return parts.sum(axis=0, dtype=np.float32) * (1.0 / SCALE)


# revision 12
# speedup vs baseline: 1.1382x; 1.1382x over previous
"""Trainium2 Bass kernel for nn_NetworkActivity_layer (masked linear):

    out = x @ (weight * mask.T).T + bias      x:(4096,15000) w:(500,15000)
                                              mask:(15000,500) bias:(500,)

Strategy: shard the contraction (gene) dim K=15000 across 8 NeuronCores
(1875 genes/core, padded to 1920 = 15 k-tiles of 128; the extra row at
gene 1875 carries the bias via an all-ones x column). Each core computes
a partial (4096,500) output; the host sums the 8 partials.

Numerics: the masked weights mw = weight * mask.T are premultiplied on
the host and scaled by 2^14 (so the fp8 tail tiles land in e4m3's normal
range); the host divides the summed output by 2^14. The first NKB
k-tiles run in bf16; the last NF k-tiles run as NF/2 fp8e4 DoubleRow
matmuls (two 128-row k-tiles per 211ns PE pass = 2x rate). Exact rel
err vs the fp32 reference on the real inputs: 1.25e-2 for NF=2,
1.75e-2 for NF=4 (gate 2e-2) — measured with a bit-exact host sim.

Per-core operands (host-packed):
  xt:  (32, 128, NKB*128) bf16  xt[m, p, k*128+c] = xpad[m*128+c, k*128+p]
       stationary lhsT slices (K=128 genes, M=128 batch), two ~2KB-row
       half loads per tile.
  gh:  (128, 6*NKB*128) bf16    the first GRP=6 m-tiles repacked
       k-tile-major: gh[p, (k*6 + m)*128 + c] = xt[m, p, k*128+c]; one
       DMA per k-tile (1.5KB rows) so arrivals match consumption order.
  x8:  (8, 128, 4*NF, 128) fp8  x8[q, p, mi*NF+2*dri+ks, c] =
       xpad[(4q+mi)*128+c, (NKB+2*dri+ks)*128+p]; DoubleRow stationary
       [K=128, ks=2, M=128] slices for 4 m-tiles per DMA.
  mw:  (128, NKB*500) bf16      mw[p, k*500+n] = mwpad[k*128+p, n];
       k0 rides the sync ring (starts earlier), k1.. on scalar, one DMA
       per k-tile so the PE never waits on a big chunk.
  mw8: (128, NF, 500) fp8       mw8[p, kk, n] = mwpad[(NKB+kk)*128+p, n]
  out: (32, 128, 500) fp16      PSUM fp32, cast on the PSUM->SBUF copy;
       host upcasts, sums, descales.

Head scheduling: the first GRP=6 m-tiles are held in 6 PSUM banks and
their matmuls are emitted k-tile-outer / m-inner, so every arriving
(mw_k, gh_k) pair unlocks 6 matmuls (1.28us of PE work per ~0.9us of
arrivals) — the whole weight load and the 1.2->2.4GHz HAM clock ramp
are absorbed by useful work. Junk matmuls on a memset tile bridge the
framework preamble to the first arrival so the ramp window never
resets. All DMAs keep >=1KB contiguous rows (sub-KB rows run ~20GB/s).
"""

import functools
import os

import ml_dtypes
import numpy as np

B, G, P = 4096, 15000, 500
N_CORES = 8
GS = G // N_CORES          # 1875 genes per core
KT = 128                   # k-tile size (partition dim; 128 enables FWL)
NK = 15                    # k-tiles per core
NF = 4                     # fp8 k-tiles (must be even); rel err 1.83e-2
NKB = NK - NF              # bf16 k-tiles
NDR = NF // 2              # DoubleRow matmuls per m-tile
KP = NK * KT               # 1920 padded genes (row GS=1875 carries bias)
MT = 128                   # batch tile
NM = B // MT               # 32 batch tiles
NQ = NM // 4               # fp8 x quads (4 m-tiles per DMA)
SCALE = np.float32(2.0 ** 14)

GRP = 6                    # m-tiles resident in PSUM during the mw load
XT_HALVES = [(0, NKB // 2 + 1), (NKB // 2 + 1, NKB)]
N_JUNK = 10                # PE clock prewarm matmuls

_BF16 = ml_dtypes.bfloat16
_F8 = ml_dtypes.float8_e4m3

LAST_EXEC_TIME_NS = None
LAST_TRACE = None
LAST_RESULTS = None


def _install_profshim():
    """Make run_bass_kernel_spmd(trace=True) work in the axon container:
    recreate the antenv.axon_hooks NTFF hook + keep artifacts local."""
    import sys
    import types

    if "antenv.axon_hooks" not in sys.modules:
        import antenv
        from trn_agent_boot.trn_boot import _ntff_profile_via_ctypes

        mod = types.ModuleType("antenv.axon_hooks")
        mod._hook = _ntff_profile_via_ctypes("/opt/axon/libaxon_pjrt.so")
        mod.set_axon_ntff_profile_hook = lambda h: setattr(mod, "_hook", h)
        mod.get_axon_ntff_profile_hook = lambda: mod._hook
        sys.modules["antenv.axon_hooks"] = mod
        antenv.axon_hooks = mod

    import concourse.bass_utils as bu

    bu.upload_artifacts = lambda tmpdir: f"file://{tmpdir}"


@functools.lru_cache(maxsize=1)
def _build():
    import concourse.bass as bass
    import concourse.mybir as mybir
    import concourse.tile as tile
    from concourse import bacc

    nc = bacc.Bacc(
        "TRN2", target_bir_lowering=False, debug=False, num_devices=N_CORES
    )
    bf16 = mybir.dt.bfloat16
    f16 = mybir.dt.float16
    f32 = mybir.dt.float32
    f8 = mybir.dt.float8e4
    DR = mybir.MatmulPerfMode.DoubleRow

    xt_d = nc.dram_tensor("xt", [NM, KT, NKB * MT], bf16, kind="ExternalInput")
    gh_d = nc.dram_tensor("gh", [KT, GRP * NKB * MT], bf16, kind="ExternalInput")
    x8_d = nc.dram_tensor("x8", [NQ, KT, 4 * NF * MT], f8, kind="ExternalInput")
    mw_d = nc.dram_tensor("mw", [KT, NKB * P], bf16, kind="ExternalInput")
    mw8_d = nc.dram_tensor("mw8", [KT, NF * P], f8, kind="ExternalInput")
    out_d = nc.dram_tensor("out", [NM, MT, P], f16, kind="ExternalOutput")

    with tile.TileContext(nc) as tc:
        with (
            tc.tile_pool(name="jpool", bufs=1) as jpool,
            tc.tile_pool(name="wpool", bufs=1) as wpool,
            tc.tile_pool(name="xpool", bufs=1) as xpool,
            tc.tile_pool(name="opool", bufs=8) as opool,
            tc.tile_pool(name="pspool", bufs=1, space=bass.MemorySpace.PSUM) as pspool,
        ):
            # Clock prewarm: PE activity right after the framework preamble
            # so the 1.2->2.4GHz ramp window elapses while the first mw
            # k-tile + gh k-tile are still in flight.
            junk = jpool.tile([KT, 512], bf16)
            nc.gpsimd.memset(junk[:], 0.0)
            jps = pspool.tile([MT, 512], f32, tag="ps", bufs=2)
            for _ in range(N_JUNK):
                nc.tensor.matmul(jps[:], junk[:, 0:128], junk[:], start=True, stop=True)

            # Premultiplied masked weights: one DMA per k-tile so matmul k
            # never waits on a multi-tile chunk. k0 on the sync ring (it
            # starts ~0.7us earlier than scalar); the rest on scalar.
            mw = wpool.tile([KT, NKB * P], bf16)
            nc.sync.dma_start(mw[:, 0:P], mw_d[:, 0:P])
            for k in range(1, NKB):
                nc.scalar.dma_start(
                    mw[:, k * P : (k + 1) * P], mw_d[:, k * P : (k + 1) * P]
                )
            mw8 = wpool.tile([KT, NF, P], f8)
            nc.scalar.dma_start(mw8[:, :, :], mw8_d[:, :])

            # Head group: GRP m-tiles pinned in PSUM, matmuls emitted
            # k-tile-outer / m-inner; gh is packed k-tile-major so each
            # per-k DMA lands exactly when that k's matmuls are due.
            gh = xpool.tile([KT, GRP * NKB * MT], bf16, name="gh", tag="gh", bufs=1)
            ghb = [0, 1, 2, 3, 5, 7, 10, NKB]
            for a, b in zip(ghb, ghb[1:]):
                nc.sync.dma_start(
                    gh[:, a * GRP * MT : b * GRP * MT],
                    gh_d[:, a * GRP * MT : b * GRP * MT],
                )
            x8q0 = xpool.tile([KT, 4 * NF, MT], f8, tag="x8", bufs=3, name="x8q0")
            nc.scalar.dma_start(x8q0[:, :, :], x8_d[0])
            x8q1 = xpool.tile([KT, 4 * NF, MT], f8, tag="x8", bufs=3, name="x8q1")
            nc.scalar.dma_start(x8q1[:, :, :], x8_d[1])
            gps = [
                pspool.tile([MT, P], f32, name=f"gps{m}", tag=f"gps{m}", bufs=1)
                for m in range(GRP)
            ]
            for k in range(NKB):
                for m in range(GRP):
                    nc.tensor.matmul(
                        gps[m][:],
                        gh[:, (k * GRP + m) * MT : (k * GRP + m + 1) * MT],
                        mw[:, k * P : (k + 1) * P],
                        start=(k == 0),
                        stop=False,
                    )
            for m in range(GRP):
                x8q = x8q0 if m < 4 else x8q1
                mi = m % 4
                for dri in range(NDR):
                    nc.tensor.matmul(
                        gps[m][:],
                        x8q[:, mi * NF + 2 * dri : mi * NF + 2 * dri + 2, :],
                        mw8[:, 2 * dri : 2 * dri + 2, :],
                        start=False,
                        stop=(dri == NDR - 1),
                        perf_mode=DR,
                    )
                ot = opool.tile([MT, P], f16, tag="ot", name="ot")
                nc.vector.tensor_copy(ot[:], gps[m][:])
                nc.scalar.dma_start(out_d[m], ot[:])

            # Steady state: one m-tile at a time, mw fully resident.
            x8q = x8q1
            for m in range(GRP, NM):
                if m % 4 == 0 and m // 4 >= 2:
                    x8q = xpool.tile([KT, 4 * NF, MT], f8, tag="x8", bufs=3, name="x8q")
                    nc.sync.dma_start(x8q[:, :, :], x8_d[m // 4])
                xt = xpool.tile([KT, NKB * MT], bf16, tag="xt", bufs=8, name="xt")
                for a, b in XT_HALVES:
                    nc.sync.dma_start(
                        xt[:, a * MT : b * MT], xt_d[m][:, a * MT : b * MT]
                    )
                ps = pspool.tile([MT, P], f32, tag="ps", bufs=2, name="ps")
                for k in range(NKB):
                    nc.tensor.matmul(
                        ps[:],
                        xt[:, k * MT : (k + 1) * MT],
                        mw[:, k * P : (k + 1) * P],
                        start=(k == 0),
                        stop=False,
                    )
                mi = m % 4
                for dri in range(NDR):
                    nc.tensor.matmul(
                        ps[:],
                        x8q[:, mi * NF + 2 * dri : mi * NF + 2 * dri + 2, :],
                        mw8[:, 2 * dri : 2 * dri + 2, :],
                        start=False,
                        stop=(dri == NDR - 1),
                        perf_mode=DR,
                    )
                ot = opool.tile([MT, P], f16, tag="ot", name="ot")
                nc.vector.tensor_copy(ot[:], ps[:])
                nc.scalar.dma_start(out_d[m], ot[:])
    nc.compile()
    return nc


def _pack_inputs(x, weight, mask, bias):
    """Host-side shard + pre-tile. Returns in_maps for the 8 cores."""
    xb = np.asarray(x, dtype=np.float32).astype(_BF16)  # (B, G) one cast pass
    wf = np.asarray(weight, dtype=np.float32)
    mf = np.asarray(mask, dtype=np.float32)
    bf = np.asarray(bias, dtype=np.float32)

    in_maps = []
    for core in range(N_CORES):
        g0 = core * GS
        xpad = np.zeros((B, KP), dtype=_BF16)
        xpad[:, :GS] = xb[:, g0 : g0 + GS]
        xpad[:, GS] = _BF16(1.0)  # bias column
        # bf16 k-tiles: [m, c, k, p] -> [m, p, k, c]
        xt = np.ascontiguousarray(
            xpad[:, : NKB * KT].reshape(NM, MT, NKB, KT).transpose(0, 3, 2, 1)
        ).reshape(NM, KT, NKB * MT)
        # head-group region repacked k-tile-major: [p, k, m, c]
        gh = np.ascontiguousarray(
            xt[:GRP].reshape(GRP, KT, NKB, MT).transpose(1, 2, 0, 3)
        ).reshape(KT, GRP * NKB * MT)
        # fp8 k-tiles: [q, mi, c, kk, p] -> [q, p, mi, kk, c]
        x8 = np.ascontiguousarray(
            xpad[:, NKB * KT :].astype(_F8)
            .reshape(NQ, 4, MT, NF, KT)
            .transpose(0, 4, 1, 3, 2)
        ).reshape(NQ, KT, 4 * NF * MT)

        # premultiplied masked weights, scaled into fp8 range:
        # mwpad[g, n] = w[n, g0+g] * mask[g0+g, n] * 2^14
        mwpad = np.zeros((KP, P), dtype=np.float32)
        mwpad[:GS] = wf[:, g0 : g0 + GS].T * mf[g0 : g0 + GS]
        if core == 0:
            mwpad[GS] = bf  # bias row (counted exactly once across cores)
        mwpad *= SCALE
        mwt = np.ascontiguousarray(
            mwpad[: NKB * KT].reshape(NKB, KT, P).transpose(1, 0, 2)
        ).reshape(KT, NKB * P).astype(_BF16)
        mw8 = np.ascontiguousarray(
            mwpad[NKB * KT :].reshape(NF, KT, P).transpose(1, 0, 2)
        ).reshape(KT, NF * P).astype(_F8)
        in_maps.append({"xt": xt, "x8": x8, "gh": gh, "mw": mwt, "mw8": mw8})
    return in_maps


def kernel(x, weight, mask, bias):
    global LAST_EXEC_TIME_NS, LAST_TRACE, LAST_RESULTS

    profile = bool(int(os.environ.get("KERNEL_PROFILE", "0")))
    if profile:
        _install_profshim()

    nc = _build()
    in_maps = _pack_inputs(x, weight, mask, bias)

    from concourse.bass_utils import run_bass_kernel_spmd

    tmpdir = None
    if profile:
        import tempfile

        base = os.environ.get("KERNEL_TRACE_DIR")
        if base:
            os.makedirs(base, exist_ok=True)
        tmpdir = tempfile.mkdtemp(prefix="ktrace_", dir=base)

    res = run_bass_kernel_spmd(
        nc,
        in_maps,
        core_ids=list(range(N_CORES)),
        trace=profile,
        tmpdir=tmpdir,
    )
    LAST_EXEC_TIME_NS = res.exec_time_ns
    LAST_TRACE = (
        res.instructions_and_trace[1] if res.instructions_and_trace else None
    )
    LAST_RESULTS = res

    parts = np.stack(
        [r["out"].astype(np.float32).reshape(B, P) for r in res.results]
    )
    return parts.sum(axis=0, dtype=np.float32) * (1.0 / SCALE)


# revision 13
# speedup vs baseline: 1.1428x; 1.0040x over previous
"""Trainium2 Bass kernel for nn_NetworkActivity_layer (masked linear):

    out = x @ (weight * mask.T).T + bias      x:(4096,15000) w:(500,15000)
                                              mask:(15000,500) bias:(500,)

Strategy: shard the contraction (gene) dim K=15000 across 8 NeuronCores
(1875 genes/core, padded to 1920 = 15 k-tiles of 128; the extra row at
gene 1875 carries the bias via an all-ones x column). Each core computes
a partial (4096,500) output; the host sums the 8 partials.

Numerics: the masked weights mw = weight * mask.T are premultiplied on
the host and scaled by 2^14 (so the fp8 tail tiles land in e4m3's normal
range); the host divides the summed output by 2^14. The first NKB
k-tiles run in bf16; the last NF k-tiles run as NF/2 fp8e4 DoubleRow
matmuls (two 128-row k-tiles per 211ns PE pass = 2x rate). Exact rel
err vs the fp32 reference on the real inputs: 1.25e-2 for NF=2,
1.75e-2 for NF=4 (gate 2e-2) — measured with a bit-exact host sim.

Per-core operands (host-packed):
  xt:  (32, 128, NKB*128) bf16  xt[m, p, k*128+c] = xpad[m*128+c, k*128+p]
       stationary lhsT slices (K=128 genes, M=128 batch), two ~2KB-row
       half loads per tile.
  gh:  (128, 6*NKB*128) bf16    the first GRP=6 m-tiles repacked
       k-tile-major: gh[p, (k*6 + m)*128 + c] = xt[m, p, k*128+c]; one
       DMA per k-tile (1.5KB rows) so arrivals match consumption order.
  x8:  (8, 128, 4*NF, 128) fp8  x8[q, p, mi*NF+2*dri+ks, c] =
       xpad[(4q+mi)*128+c, (NKB+2*dri+ks)*128+p]; DoubleRow stationary
       [K=128, ks=2, M=128] slices for 4 m-tiles per DMA.
  mw:  (128, NKB*500) bf16      mw[p, k*500+n] = mwpad[k*128+p, n];
       k0 rides the sync ring (starts earlier), k1.. on scalar, one DMA
       per k-tile so the PE never waits on a big chunk.
  mw8: (128, NF, 500) fp8       mw8[p, kk, n] = mwpad[(NKB+kk)*128+p, n]
  out: (32, 128, 500) fp16      PSUM fp32, cast on the PSUM->SBUF copy;
       host upcasts, sums, descales.

Head scheduling: the first GRP=6 m-tiles are held in 6 PSUM banks and
their matmuls are emitted k-tile-outer / m-inner, so every arriving
(mw_k, gh_k) pair unlocks 6 matmuls (1.28us of PE work per ~0.9us of
arrivals) — the whole weight load and the 1.2->2.4GHz HAM clock ramp
are absorbed by useful work. Junk matmuls on a memset tile bridge the
framework preamble to the first arrival so the ramp window never
resets. All DMAs keep >=1KB contiguous rows (sub-KB rows run ~20GB/s).
"""

import functools
import os

import ml_dtypes
import numpy as np

B, G, P = 4096, 15000, 500
N_CORES = 8
GS = G // N_CORES          # 1875 genes per core
KT = 128                   # k-tile size (partition dim; 128 enables FWL)
NK = 15                    # k-tiles per core
NF = 4                     # fp8 k-tiles (must be even); rel err 1.83e-2
NKB = NK - NF              # bf16 k-tiles
NDR = NF // 2              # DoubleRow matmuls per m-tile
KP = NK * KT               # 1920 padded genes (row GS=1875 carries bias)
MT = 128                   # batch tile
NM = B // MT               # 32 batch tiles
NQ = NM // 4               # fp8 x quads (4 m-tiles per DMA)
SCALE = np.float32(2.0 ** 14)

GRP = 6                    # m-tiles resident in PSUM during the mw load
XT_HALVES = [(0, NKB // 2 + 1), (NKB // 2 + 1, NKB)]
N_JUNK = 10                # PE clock prewarm matmuls

_BF16 = ml_dtypes.bfloat16
_F8 = ml_dtypes.float8_e4m3

LAST_EXEC_TIME_NS = None
LAST_TRACE = None
LAST_RESULTS = None


def _install_profshim():
    """Make run_bass_kernel_spmd(trace=True) work in the axon container:
    recreate the antenv.axon_hooks NTFF hook + keep artifacts local."""
    import sys
    import types

    if "antenv.axon_hooks" not in sys.modules:
        import antenv
        from trn_agent_boot.trn_boot import _ntff_profile_via_ctypes

        mod = types.ModuleType("antenv.axon_hooks")
        mod._hook = _ntff_profile_via_ctypes("/opt/axon/libaxon_pjrt.so")
        mod.set_axon_ntff_profile_hook = lambda h: setattr(mod, "_hook", h)
        mod.get_axon_ntff_profile_hook = lambda: mod._hook
        sys.modules["antenv.axon_hooks"] = mod
        antenv.axon_hooks = mod

    import concourse.bass_utils as bu

    bu.upload_artifacts = lambda tmpdir: f"file://{tmpdir}"


@functools.lru_cache(maxsize=1)
def _build():
    import concourse.bass as bass
    import concourse.mybir as mybir
    import concourse.tile as tile
    from concourse import bacc

    nc = bacc.Bacc(
        "TRN2", target_bir_lowering=False, debug=False, num_devices=N_CORES
    )
    bf16 = mybir.dt.bfloat16
    f16 = mybir.dt.float16
    f32 = mybir.dt.float32
    f8 = mybir.dt.float8e4
    DR = mybir.MatmulPerfMode.DoubleRow

    xt_d = nc.dram_tensor("xt", [NM, KT, NKB * MT], bf16, kind="ExternalInput")
    gh_d = nc.dram_tensor("gh", [KT, GRP * NKB * MT], bf16, kind="ExternalInput")
    x8_d = nc.dram_tensor("x8", [NQ, KT, 4 * NF * MT], f8, kind="ExternalInput")
    mw_d = nc.dram_tensor("mw", [KT, NKB * P], bf16, kind="ExternalInput")
    mw8_d = nc.dram_tensor("mw8", [KT, NF * P], f8, kind="ExternalInput")
    out_d = nc.dram_tensor("out", [NM, MT, P], f16, kind="ExternalOutput")

    with tile.TileContext(nc) as tc:
        with (
            tc.tile_pool(name="jpool", bufs=1) as jpool,
            tc.tile_pool(name="wpool", bufs=1) as wpool,
            tc.tile_pool(name="xpool", bufs=1) as xpool,
            tc.tile_pool(name="opool", bufs=8) as opool,
            tc.tile_pool(name="pspool", bufs=1, space=bass.MemorySpace.PSUM) as pspool,
        ):
            # Clock prewarm: PE activity right after the framework preamble
            # so the 1.2->2.4GHz ramp window elapses while the first mw
            # k-tile + gh k-tile are still in flight.
            junk = jpool.tile([KT, 512], bf16)
            nc.gpsimd.memset(junk[:], 0.0)
            jps = pspool.tile([MT, 512], f32, tag="ps", bufs=2)
            for _ in range(N_JUNK):
                nc.tensor.matmul(jps[:], junk[:, 0:128], junk[:], start=True, stop=True)

            # Premultiplied masked weights: one DMA per k-tile so matmul k
            # never waits on a multi-tile chunk. k0 on the sync ring (it
            # starts ~0.7us earlier than scalar); the rest on scalar.
            mw = wpool.tile([KT, NKB * P], bf16)
            nc.sync.dma_start(mw[:, 0:P], mw_d[:, 0:P])
            for k in range(1, NKB):
                nc.scalar.dma_start(
                    mw[:, k * P : (k + 1) * P], mw_d[:, k * P : (k + 1) * P]
                )
            mw8 = wpool.tile([KT, NF, P], f8)
            nc.scalar.dma_start(mw8[:, :, :], mw8_d[:, :])

            # Head group: GRP m-tiles pinned in PSUM, matmuls emitted
            # k-tile-outer / m-inner; gh is packed k-tile-major so each
            # per-k DMA lands exactly when that k's matmuls are due.
            gh = xpool.tile([KT, GRP * NKB * MT], bf16, name="gh", tag="gh", bufs=1)
            ghb = [0, 1, 2, 3, 4, 5, 7, 9, NKB]
            for a, b in zip(ghb, ghb[1:]):
                nc.sync.dma_start(
                    gh[:, a * GRP * MT : b * GRP * MT],
                    gh_d[:, a * GRP * MT : b * GRP * MT],
                )
            x8q0 = xpool.tile([KT, 4 * NF, MT], f8, tag="x8", bufs=3, name="x8q0")
            nc.scalar.dma_start(x8q0[:, :, :], x8_d[0])
            x8q1 = xpool.tile([KT, 4 * NF, MT], f8, tag="x8", bufs=3, name="x8q1")
            nc.scalar.dma_start(x8q1[:, :, :], x8_d[1])
            gps = [
                pspool.tile([MT, P], f32, name=f"gps{m}", tag=f"gps{m}", bufs=1)
                for m in range(GRP)
            ]
            for k in range(NKB):
                for m in range(GRP):
                    nc.tensor.matmul(
                        gps[m][:],
                        gh[:, (k * GRP + m) * MT : (k * GRP + m + 1) * MT],
                        mw[:, k * P : (k + 1) * P],
                        start=(k == 0),
                        stop=False,
                    )
            for m in range(GRP):
                x8q = x8q0 if m < 4 else x8q1
                mi = m % 4
                for dri in range(NDR):
                    nc.tensor.matmul(
                        gps[m][:],
                        x8q[:, mi * NF + 2 * dri : mi * NF + 2 * dri + 2, :],
                        mw8[:, 2 * dri : 2 * dri + 2, :],
                        start=False,
                        stop=(dri == NDR - 1),
                        perf_mode=DR,
                    )
                ot = opool.tile([MT, P], f16, tag="ot", name="ot")
                nc.vector.tensor_copy(ot[:], gps[m][:])
                nc.scalar.dma_start(out_d[m], ot[:])

            # Steady state: one m-tile at a time, mw fully resident.
            x8q = x8q1
            for m in range(GRP, NM):
                if m % 4 == 0 and m // 4 >= 2:
                    x8q = xpool.tile([KT, 4 * NF, MT], f8, tag="x8", bufs=3, name="x8q")
                    nc.sync.dma_start(x8q[:, :, :], x8_d[m // 4])
                xt = xpool.tile([KT, NKB * MT], bf16, tag="xt", bufs=8, name="xt")
                for a, b in XT_HALVES:
                    nc.sync.dma_start(
                        xt[:, a * MT : b * MT], xt_d[m][:, a * MT : b * MT]
                    )
                ps = pspool.tile([MT, P], f32, tag="ps", bufs=2, name="ps")
                for k in range(NKB):
                    nc.tensor.matmul(
                        ps[:],
                        xt[:, k * MT : (k + 1) * MT],
                        mw[:, k * P : (k + 1) * P],
                        start=(k == 0),
                        stop=False,
                    )
                mi = m % 4
                for dri in range(NDR):
                    nc.tensor.matmul(
                        ps[:],
                        x8q[:, mi * NF + 2 * dri : mi * NF + 2 * dri + 2, :],
                        mw8[:, 2 * dri : 2 * dri + 2, :],
                        start=False,
                        stop=(dri == NDR - 1),
                        perf_mode=DR,
                    )
                ot = opool.tile([MT, P], f16, tag="ot", name="ot")
                nc.vector.tensor_copy(ot[:], ps[:])
                # last tile's store rides the (by then idle) sync ring to
                # shorten the end-of-kernel drain
                eng = nc.sync if m == NM - 1 else nc.scalar
                eng.dma_start(out_d[m], ot[:])
    nc.compile()
    return nc


def _pack_inputs(x, weight, mask, bias):
    """Host-side shard + pre-tile. Returns in_maps for the 8 cores."""
    xb = np.asarray(x, dtype=np.float32).astype(_BF16)  # (B, G) one cast pass
    wf = np.asarray(weight, dtype=np.float32)
    mf = np.asarray(mask, dtype=np.float32)
    bf = np.asarray(bias, dtype=np.float32)

    in_maps = []
    for core in range(N_CORES):
        g0 = core * GS
        xpad = np.zeros((B, KP), dtype=_BF16)
        xpad[:, :GS] = xb[:, g0 : g0 + GS]
        xpad[:, GS] = _BF16(1.0)  # bias column
        # bf16 k-tiles: [m, c, k, p] -> [m, p, k, c]
        xt = np.ascontiguousarray(
            xpad[:, : NKB * KT].reshape(NM, MT, NKB, KT).transpose(0, 3, 2, 1)
        ).reshape(NM, KT, NKB * MT)
        # head-group region repacked k-tile-major: [p, k, m, c]
        gh = np.ascontiguousarray(
            xt[:GRP].reshape(GRP, KT, NKB, MT).transpose(1, 2, 0, 3)
        ).reshape(KT, GRP * NKB * MT)
        # fp8 k-tiles: [q, mi, c, kk, p] -> [q, p, mi, kk, c]
        x8 = np.ascontiguousarray(
            xpad[:, NKB * KT :].astype(_F8)
            .reshape(NQ, 4, MT, NF, KT)
            .transpose(0, 4, 1, 3, 2)
        ).reshape(NQ, KT, 4 * NF * MT)

        # premultiplied masked weights, scaled into fp8 range:
        # mwpad[g, n] = w[n, g0+g] * mask[g0+g, n] * 2^14
        mwpad = np.zeros((KP, P), dtype=np.float32)
        mwpad[:GS] = wf[:, g0 : g0 + GS].T * mf[g0 : g0 + GS]
        if core == 0:
            mwpad[GS] = bf  # bias row (counted exactly once across cores)
        mwpad *= SCALE
        mwt = np.ascontiguousarray(
            mwpad[: NKB * KT].reshape(NKB, KT, P).transpose(1, 0, 2)
        ).reshape(KT, NKB * P).astype(_BF16)
        mw8 = np.ascontiguousarray(
            mwpad[NKB * KT :].reshape(NF, KT, P).transpose(1, 0, 2)
        ).reshape(KT, NF * P).astype(_F8)
        in_maps.append({"xt": xt, "x8": x8, "gh": gh, "mw": mwt, "mw8": mw8})
    return in_maps


def kernel(x, weight, mask, bias):
    global LAST_EXEC_TIME_NS, LAST_TRACE, LAST_RESULTS

    profile = bool(int(os.environ.get("KERNEL_PROFILE", "0")))
    if profile:
        _install_profshim()

    nc = _build()
    in_maps = _pack_inputs(x, weight, mask, bias)

    from concourse.bass_utils import run_bass_kernel_spmd

    tmpdir = None
    if profile:
        import tempfile

        base = os.environ.get("KERNEL_TRACE_DIR")
        if base:
            os.makedirs(base, exist_ok=True)
        tmpdir = tempfile.mkdtemp(prefix="ktrace_", dir=base)

    res = run_bass_kernel_spmd(
        nc,
        in_maps,
        core_ids=list(range(N_CORES)),
        trace=profile,
        tmpdir=tmpdir,
    )
    LAST_EXEC_TIME_NS = res.exec_time_ns
    LAST_TRACE = (
        res.instructions_and_trace[1] if res.instructions_and_trace else None
    )
    LAST_RESULTS = res

    parts = np.stack(
        [r["out"].astype(np.float32).reshape(B, P) for r in res.results]
    )
    return parts.sum(axis=0, dtype=np.float32) * (1.0 / SCALE)


# revision 19
# speedup vs baseline: 1.1478x; 1.0044x over previous
"""Trainium2 Bass kernel for nn_NetworkActivity_layer (masked linear):

    out = x @ (weight * mask.T).T + bias      x:(4096,15000) w:(500,15000)
                                              mask:(15000,500) bias:(500,)

Strategy: shard the contraction (gene) dim K=15000 across 8 NeuronCores
(1875 genes/core, padded to 1920 = 15 k-tiles of 128; the extra row at
gene 1875 carries the bias via an all-ones x column). Each core computes
a partial (4096,500) output; the host sums the 8 partials.

Numerics: the masked weights mw = weight * mask.T are premultiplied on
the host and scaled by 2^14 (so the fp8 tail tiles land in e4m3's normal
range); the host divides the summed output by 2^14. The first NKB
k-tiles run in bf16; the last NF k-tiles run as NF/2 fp8e4 DoubleRow
matmuls (two 128-row k-tiles per 211ns PE pass = 2x rate). Exact rel
err vs the fp32 reference on the real inputs: 1.25e-2 for NF=2,
1.75e-2 for NF=4 (gate 2e-2) — measured with a bit-exact host sim.

Per-core operands (host-packed):
  xt:  (32, 128, NKB*128) bf16  xt[m, p, k*128+c] = xpad[m*128+c, k*128+p]
       stationary lhsT slices (K=128 genes, M=128 batch), two ~2KB-row
       half loads per tile.
  gh:  (128, 6*NKB*128) bf16    the first GRP=6 m-tiles repacked
       k-tile-major: gh[p, (k*6 + m)*128 + c] = xt[m, p, k*128+c]; one
       DMA per k-tile (1.5KB rows) so arrivals match consumption order.
  x8:  (8, 128, 4*NF, 128) fp8  x8[q, p, mi*NF+2*dri+ks, c] =
       xpad[(4q+mi)*128+c, (NKB+2*dri+ks)*128+p]; DoubleRow stationary
       [K=128, ks=2, M=128] slices for 4 m-tiles per DMA.
  mw:  (128, NKB*500) bf16      mw[p, k*500+n] = mwpad[k*128+p, n];
       k0 rides the sync ring (starts earlier), k1.. on scalar, one DMA
       per k-tile so the PE never waits on a big chunk.
  mw8: (128, NF, 500) fp8       mw8[p, kk, n] = mwpad[(NKB+kk)*128+p, n]
  out: (32, 128, 500) fp16      PSUM fp32, cast on the PSUM->SBUF copy;
       host upcasts, sums, descales.

Head scheduling: the first GRP=6 m-tiles are held in 6 PSUM banks and
their matmuls are emitted k-tile-outer / m-inner, so every arriving
(mw_k, gh_k) pair unlocks 6 matmuls (1.28us of PE work per ~0.9us of
arrivals) — the whole weight load and the 1.2->2.4GHz HAM clock ramp
are absorbed by useful work. Junk matmuls on a memset tile bridge the
framework preamble to the first arrival so the ramp window never
resets. All DMAs keep >=1KB contiguous rows (sub-KB rows run ~20GB/s).
"""

import functools
import os

import ml_dtypes
import numpy as np

B, G, P = 4096, 15000, 500
N_CORES = 8
GS = G // N_CORES          # 1875 genes per core
KT = 128                   # k-tile size (partition dim; 128 enables FWL)
NK = 15                    # k-tiles per core
NF = 4                     # fp8 k-tiles (must be even); rel err 1.83e-2
NKB = NK - NF              # bf16 k-tiles
NDR = NF // 2              # DoubleRow matmuls per m-tile
KP = NK * KT               # 1920 padded genes (row GS=1875 carries bias)
MT = 128                   # batch tile
NM = B // MT               # 32 batch tiles
NQ = NM // 4               # fp8 x quads (4 m-tiles per DMA)
SCALE = np.float32(2.0 ** 14)

GRP = 6                    # m-tiles resident in PSUM during the mw load
XT_HALVES = [(0, NKB // 2 + 1), (NKB // 2 + 1, NKB)]
N_JUNK = 10                # PE clock prewarm matmuls

_BF16 = ml_dtypes.bfloat16
_F8 = ml_dtypes.float8_e4m3

LAST_EXEC_TIME_NS = None
LAST_TRACE = None
LAST_RESULTS = None


def _install_profshim():
    """Make run_bass_kernel_spmd(trace=True) work in the axon container:
    recreate the antenv.axon_hooks NTFF hook + keep artifacts local."""
    import sys
    import types

    if "antenv.axon_hooks" not in sys.modules:
        import antenv
        from trn_agent_boot.trn_boot import _ntff_profile_via_ctypes

        mod = types.ModuleType("antenv.axon_hooks")
        mod._hook = _ntff_profile_via_ctypes("/opt/axon/libaxon_pjrt.so")
        mod.set_axon_ntff_profile_hook = lambda h: setattr(mod, "_hook", h)
        mod.get_axon_ntff_profile_hook = lambda: mod._hook
        sys.modules["antenv.axon_hooks"] = mod
        antenv.axon_hooks = mod

    import concourse.bass_utils as bu

    bu.upload_artifacts = lambda tmpdir: f"file://{tmpdir}"


@functools.lru_cache(maxsize=1)
def _build():
    import concourse.bass as bass
    import concourse.mybir as mybir
    import concourse.tile as tile
    from concourse import bacc

    nc = bacc.Bacc(
        "TRN2", target_bir_lowering=False, debug=False, num_devices=N_CORES
    )
    bf16 = mybir.dt.bfloat16
    f16 = mybir.dt.float16
    f32 = mybir.dt.float32
    f8 = mybir.dt.float8e4
    DR = mybir.MatmulPerfMode.DoubleRow

    xt_d = nc.dram_tensor("xt", [NM, KT, NKB * MT], bf16, kind="ExternalInput")
    gh_d = nc.dram_tensor("gh", [KT, GRP * NKB * MT], bf16, kind="ExternalInput")
    x8_d = nc.dram_tensor("x8", [NQ, KT, 4 * NF * MT], f8, kind="ExternalInput")
    mw_d = nc.dram_tensor("mw", [KT, NKB * P], bf16, kind="ExternalInput")
    mw8_d = nc.dram_tensor("mw8", [KT, NF * P], f8, kind="ExternalInput")
    out_d = nc.dram_tensor("out", [NM, MT, P], f16, kind="ExternalOutput")

    with tile.TileContext(nc) as tc:
        with (
            tc.tile_pool(name="jpool", bufs=1) as jpool,
            tc.tile_pool(name="wpool", bufs=1) as wpool,
            tc.tile_pool(name="xpool", bufs=1) as xpool,
            tc.tile_pool(name="opool", bufs=8) as opool,
            tc.tile_pool(name="pspool", bufs=1, space=bass.MemorySpace.PSUM) as pspool,
        ):
            # Clock prewarm: PE activity right after the framework preamble
            # so the 1.2->2.4GHz ramp window elapses while the first mw
            # k-tile + gh k-tile are still in flight.
            junk = jpool.tile([KT, 512], bf16)
            nc.gpsimd.memset(junk[:], 0.0)
            jps = pspool.tile([MT, 512], f32, tag="ps", bufs=2)
            for _ in range(N_JUNK):
                nc.tensor.matmul(jps[:], junk[:, 0:128], junk[:], start=True, stop=True)

            # Premultiplied masked weights: one DMA per k-tile so matmul k
            # never waits on a multi-tile chunk. k0 on the sync ring (it
            # starts ~0.7us earlier than scalar); the rest on scalar.
            mw = wpool.tile([KT, NKB * P], bf16)
            nc.sync.dma_start(mw[:, 0:P], mw_d[:, 0:P])
            for k in range(1, NKB):
                nc.scalar.dma_start(
                    mw[:, k * P : (k + 1) * P], mw_d[:, k * P : (k + 1) * P]
                )
            mw8 = wpool.tile([KT, NF, P], f8)
            nc.scalar.dma_start(mw8[:, :, :], mw8_d[:, :])

            # Head group: GRP m-tiles pinned in PSUM, matmuls emitted
            # k-tile-outer / m-inner; gh is packed k-tile-major so each
            # per-k DMA lands exactly when that k's matmuls are due.
            gh = xpool.tile([KT, GRP * NKB * MT], bf16, name="gh", tag="gh", bufs=1)
            ghb = list(range(NKB + 1))
            for a, b in zip(ghb, ghb[1:]):
                nc.sync.dma_start(
                    gh[:, a * GRP * MT : b * GRP * MT],
                    gh_d[:, a * GRP * MT : b * GRP * MT],
                )
            x8q0 = xpool.tile([KT, 4 * NF, MT], f8, tag="x8", bufs=3, name="x8q0")
            nc.scalar.dma_start(x8q0[:, :, :], x8_d[0])
            x8q1 = xpool.tile([KT, 4 * NF, MT], f8, tag="x8", bufs=3, name="x8q1")
            nc.scalar.dma_start(x8q1[:, :, :], x8_d[1])
            gps = [
                pspool.tile([MT, P], f32, name=f"gps{m}", tag=f"gps{m}", bufs=1)
                for m in range(GRP)
            ]
            for k in range(NKB):
                for m in range(GRP):
                    nc.tensor.matmul(
                        gps[m][:],
                        gh[:, (k * GRP + m) * MT : (k * GRP + m + 1) * MT],
                        mw[:, k * P : (k + 1) * P],
                        start=(k == 0),
                        stop=False,
                    )
            for m in range(GRP):
                x8q = x8q0 if m < 4 else x8q1
                mi = m % 4
                for dri in range(NDR):
                    nc.tensor.matmul(
                        gps[m][:],
                        x8q[:, mi * NF + 2 * dri : mi * NF + 2 * dri + 2, :],
                        mw8[:, 2 * dri : 2 * dri + 2, :],
                        start=False,
                        stop=(dri == NDR - 1),
                        perf_mode=DR,
                    )
                ot = opool.tile([MT, P], f16, tag="ot", name="ot")
                nc.vector.tensor_copy(ot[:], gps[m][:])
                nc.scalar.dma_start(out_d[m], ot[:])

            # Steady state: one m-tile at a time, mw fully resident.
            x8q = x8q1
            for m in range(GRP, NM):
                if m % 4 == 0 and m // 4 >= 2:
                    x8q = xpool.tile([KT, 4 * NF, MT], f8, tag="x8", bufs=3, name="x8q")
                    nc.sync.dma_start(x8q[:, :, :], x8_d[m // 4])
                xt = xpool.tile([KT, NKB * MT], bf16, tag="xt", bufs=8, name="xt")
                nc.sync.dma_start(xt[:], xt_d[m])
                ps = pspool.tile([MT, P], f32, tag="ps", bufs=2, name="ps")
                for k in range(NKB):
                    nc.tensor.matmul(
                        ps[:],
                        xt[:, k * MT : (k + 1) * MT],
                        mw[:, k * P : (k + 1) * P],
                        start=(k == 0),
                        stop=False,
                    )
                mi = m % 4
                for dri in range(NDR):
                    nc.tensor.matmul(
                        ps[:],
                        x8q[:, mi * NF + 2 * dri : mi * NF + 2 * dri + 2, :],
                        mw8[:, 2 * dri : 2 * dri + 2, :],
                        start=False,
                        stop=(dri == NDR - 1),
                        perf_mode=DR,
                    )
                ot = opool.tile([MT, P], f16, tag="ot", name="ot")
                nc.vector.tensor_copy(ot[:], ps[:])
                # last tile's store rides the (by then idle) sync ring to
                # shorten the end-of-kernel drain
                eng = nc.sync if m == NM - 1 else nc.scalar
                eng.dma_start(out_d[m], ot[:])
    nc.compile()
    return nc


def _pack_inputs(x, weight, mask, bias):
    """Host-side shard + pre-tile. Returns in_maps for the 8 cores."""
    xb = np.asarray(x, dtype=np.float32).astype(_BF16)  # (B, G) one cast pass
    wf = np.asarray(weight, dtype=np.float32)
    mf = np.asarray(mask, dtype=np.float32)
    bf = np.asarray(bias, dtype=np.float32)

    in_maps = []
    for core in range(N_CORES):
        g0 = core * GS
        xpad = np.zeros((B, KP), dtype=_BF16)
        xpad[:, :GS] = xb[:, g0 : g0 + GS]
        xpad[:, GS] = _BF16(1.0)  # bias column
        # bf16 k-tiles: [m, c, k, p] -> [m, p, k, c]
        xt = np.ascontiguousarray(
            xpad[:, : NKB * KT].reshape(NM, MT, NKB, KT).transpose(0, 3, 2, 1)
        ).reshape(NM, KT, NKB * MT)
        # head-group region repacked k-tile-major: [p, k, m, c]
        gh = np.ascontiguousarray(
            xt[:GRP].reshape(GRP, KT, NKB, MT).transpose(1, 2, 0, 3)
        ).reshape(KT, GRP * NKB * MT)
        # fp8 k-tiles: [q, mi, c, kk, p] -> [q, p, mi, kk, c]
        x8 = np.ascontiguousarray(
            xpad[:, NKB * KT :].astype(_F8)
            .reshape(NQ, 4, MT, NF, KT)
            .transpose(0, 4, 1, 3, 2)
        ).reshape(NQ, KT, 4 * NF * MT)

        # premultiplied masked weights, scaled into fp8 range:
        # mwpad[g, n] = w[n, g0+g] * mask[g0+g, n] * 2^14
        mwpad = np.zeros((KP, P), dtype=np.float32)
        mwpad[:GS] = wf[:, g0 : g0 + GS].T * mf[g0 : g0 + GS]
        if core == 0:
            mwpad[GS] = bf  # bias row (counted exactly once across cores)
        mwpad *= SCALE
        mwt = np.ascontiguousarray(
            mwpad[: NKB * KT].reshape(NKB, KT, P).transpose(1, 0, 2)
        ).reshape(KT, NKB * P).astype(_BF16)
        mw8 = np.ascontiguousarray(
            mwpad[NKB * KT :].reshape(NF, KT, P).transpose(1, 0, 2)
        ).reshape(KT, NF * P).astype(_F8)
        in_maps.append({"xt": xt, "x8": x8, "gh": gh, "mw": mwt, "mw8": mw8})
    return in_maps


def kernel(x, weight, mask, bias):
    global LAST_EXEC_TIME_NS, LAST_TRACE, LAST_RESULTS

    profile = bool(int(os.environ.get("KERNEL_PROFILE", "0")))
    if profile:
        _install_profshim()

    nc = _build()
    in_maps = _pack_inputs(x, weight, mask, bias)

    from concourse.bass_utils import run_bass_kernel_spmd

    tmpdir = None
    if profile:
        import tempfile

        base = os.environ.get("KERNEL_TRACE_DIR")
        if base:
            os.makedirs(base, exist_ok=True)
        tmpdir = tempfile.mkdtemp(prefix="ktrace_", dir=base)

    res = run_bass_kernel_spmd(
        nc,
        in_maps,
        core_ids=list(range(N_CORES)),
        trace=profile,
        tmpdir=tmpdir,
    )
    LAST_EXEC_TIME_NS = res.exec_time_ns
    LAST_TRACE = (
        res.instructions_and_trace[1] if res.instructions_and_trace else None
    )
    LAST_RESULTS = res

    parts = np.stack(
        [r["out"].astype(np.float32).reshape(B, P) for r in res.results]
    )
    return parts.sum(axis=0, dtype=np.float32) * (1.0 / SCALE)
